# revision 17
# baseline (speedup 1.0000x reference)
"""Trainium2 Bass kernel: per-molecule Gram min-eigenvalue (segment_reduce).

Problem: scalar_representation [131072, 128] f32, idx_m [131072] i32 (sorted,
uniform 16-atom segments -> 8192 molecules). For each molecule m:
  C_m = T_m @ T_m.T (16x16 PSD Gram), output lambda_min(C_m). Out: [8192] f32.

Sharding: data-parallel over molecules, 8 contiguous row-blocks of 16384 rows
(1024 molecules) per NeuronCore. No cross-core communication.

Per-core pipeline:
  phase 1 (per 128-row tile t, 128 tiles):
    DMA load X_t [128,128] -> PE transpose -> ACT copy -> XT [feat, rows]
    4x PE matmul (pair g of 2 mols: stationary/moving = XT cols 32g:32g+32)
      accumulated side by side into one PSUM bank [32, 128]
    copy PSUM->SBUF (alternating ACT/DVE), 2 extraction DMAs -> E
  E layout: partition t holds tile t's 8 molecules: [128, 8*256] (mol=(t,m),
    C entries (i,j) at free offset m*256 + i*16 + j)
  phase 2 (batched eigensolve on E, all ops elementwise over molecules):
    scale C by 1/s (s=trace/16) -> Householder tridiag (14 steps, shrinking)
    -> lambda_min via Sturm-grid localization + Laguerre iterations on the
    char-poly 3-term recurrence -> rescale -> DMA out.
"""
import sys
import dataclasses

import numpy as np

sys.path.insert(0, "/opt/trn_rl_repo")

import concourse.bass as bass  # noqa: E402
import concourse.mybir as mybir  # noqa: E402
import concourse.tile as tile  # noqa: E402
from concourse.masks import make_identity  # noqa: E402

F32 = mybir.dt.float32
ALU = mybir.AluOpType
ACTF = mybir.ActivationFunctionType
AX = mybir.AxisListType

N_CORES = 8
FEAT = 128
NA = 16                      # atoms per molecule
ROWS_PER_CORE = 131072 // N_CORES   # 16384
N_TILES_FULL = ROWS_PER_CORE // 128  # 128
NX = 16                      # Sturm grid points
N_LAG = 4                    # Laguerre iterations
EPS_GUARD = 1e-30
P_GUARD = 1e-35


def _re(ap, new_ap, extra_off=0):
    """Rebuild an AP with explicit [step, num] dims (element units)."""
    return dataclasses.replace(ap, ap=new_ap, offset=ap.offset + extra_off)


def build_core_kernel(tc, x_ap, out_ap, n_tiles):
    """Emit the per-core kernel. x: [n_tiles*128, 128] f32, out: [n_tiles*8]."""
    nc = tc.nc
    G = (n_tiles * 8) // 128          # molecules per partition (slots)
    assert n_tiles * 8 == G * 128, "n_tiles must be a multiple of 16"
    assert G in (1, 8), "extraction scheme supports G=1 (test) or G=8 (full)"
    n = float(NA)

    ctx = tc._kernel_ctx
    consts = ctx.enter_context(tc.tile_pool(name="consts", bufs=1))
    # bufs=8 on DMA-written/read pools: slot-WAW then lands on the same
    # DMAHW lane as the queue-reuse wait (3 DMAs/tile x 8 lanes), merging
    # two waits -> stays within the DMACopy sync-wait limit.
    loadp = ctx.enter_context(tc.tile_pool(name="loadp", bufs=8))
    xtp = ctx.enter_context(tc.tile_pool(name="xtp", bufs=4))
    grsp = ctx.enter_context(tc.tile_pool(name="grsp", bufs=8))
    pst = ctx.enter_context(tc.tile_pool(name="pst", bufs=2, space="PSUM"))
    psg = ctx.enter_context(tc.tile_pool(name="psg", bufs=2, space="PSUM"))
    eig = ctx.enter_context(tc.tile_pool(name="eig", bufs=1))

    ident = consts.tile([128, 128], F32)
    make_identity(nc, ident)
    # PE matmuls can carry only ONE sync wait in codegen. A dummy first PE op
    # absorbs the wait on the gpsimd-written identity, so every later PE op
    # needs just one wait (all other deps ride the ACT semaphore, see below).
    dummy_pool = ctx.enter_context(tc.tile_pool(name="dummy_pool", bufs=1,
                                                space="PSUM"))
    dummy_ps = dummy_pool.tile([128, 128], F32)
    pe_first = nc.tensor.transpose(dummy_ps[:, :], ident[:, :], ident[:, :])
    dummy_sb = consts.tile([128, 1], F32)
    nc.scalar.copy(dummy_sb[:, :], dummy_ps[:, 0:1])

    # ---- grid fractions const [128, NX]: (j+1)/(NX+1) ----
    fr = consts.tile([128, NX], F32)
    nc.gpsimd.iota(fr[:, :], [[1, NX]], channel_multiplier=0,
                   allow_small_or_imprecise_dtypes=True)
    nc.vector.tensor_scalar(fr[:, :], fr[:, :], 1.0, 1.0 / (NX + 1.0),
                            ALU.add, ALU.mult)
    # ---- Laguerre addend multipliers [128, 2G]: (1.0)*G, (2.0)*G ----
    k2 = consts.tile([128, 2 * G], F32)
    nc.vector.memset(k2[:, 0:G], 1.0)
    nc.vector.memset(k2[:, G:2 * G], 2.0)

    # ---- E: eigensolve working tile; E_raw: extraction staging (G>=2) ----
    E = eig.tile([128, G * 256], F32)
    Erow = G * 256
    if G >= 2:
        E_raw = eig.tile([128, G * 256], F32, tag="E_raw")
    else:
        E_raw = E

    # ================= phase 1: load, transpose, gram, extract =============
    from concourse.tile_rust import add_dep_helper
    for t in range(n_tiles):
        xnat = loadp.tile([128, 128], F32, tag="xnat")
        nc.sync.dma_start(xnat[:, :], x_ap[t * 128:(t + 1) * 128, :])

        ps1 = pst.tile([128, 128], F32, tag="ps_t")
        tr = nc.tensor.transpose(ps1[:, :], xnat[:, :], ident[:, :])
        if t == 0:
            # tr depends on pe_first (arg0 waits on arg1)
            add_dep_helper(tr.ins, pe_first.ins, sync=False,
                           reason="dummy PE op first (absorb Pool wait)")
        xt = xtp.tile([128, 128], F32, tag="xt")
        nc.scalar.copy(xt[:, :], ps1[:, :])

        ps2 = psg.tile([32, 128], F32, tag="ps_g")
        for q in range(4):
            nc.tensor.matmul(ps2[:, 32 * q:32 * (q + 1)],
                             xt[:, 32 * q:32 * (q + 1)],
                             xt[:, 32 * q:32 * (q + 1)],
                             start=(q == 0), stop=(q == 3),
                             tile_position=(0, 0))
        # ACT for ALL psum->sbuf copies: every PE op's deps then collapse onto
        # the single ACT semaphore (older psum-release ticks are covered by the
        # same-tile ACT wait and elided) -> one sync wait per matmul.
        gr = grsp.tile([32, 128], F32, tag="gr")
        nc.scalar.copy(gr[:, :], ps2[:, :])

        # extraction. gr diag block of mol m=2g+e at partitions 16e+i, cols 32g+16e+j.
        grap = gr[:, :]
        if G >= 2:
            # dst partition t, contiguous (layout (e, i, g, j)); src iter (i, g, j)
            for e in range(2):
                src = _re(grap, [[128, 16], [32, 4], [1, 16]], e * (16 * 128 + 16))
                dst = E_raw[t:t + 1, e * 1024:(e + 1) * 1024]
                nc.sync.dma_start(dst, src)
        else:
            # G==1 (small test): one DMA per molecule; dst partition t*8+m,
            # contiguous 256 = (i, j) = E layout (i, m=1, j) directly.
            for m in range(8):
                e, g = m % 2, m // 2
                src = _re(grap, [[128, 16], [1, 16]], e * (16 * 128 + 16) + 32 * g)
                dst = E[t * 8 + m:t * 8 + m + 1, :]
                nc.sync.dma_start(dst, src)

    if G >= 2:
        # reorder E_raw (e, i, g, j) -> E (i, m=2g+e, j): two strided copies
        for e in range(2):
            src = _re(E_raw[:, :], [[Erow, 128], [64, 16], [16, 4], [1, 16]],
                      e * 1024)
            dst = _re(E[:, :], [[Erow, 128], [16 * G, 16], [32, 4], [1, 16]],
                      16 * e)
            nc.vector.tensor_copy(dst, src)

    # ================= phase 2: batched eigensolve =========================
    # scratch tiles
    s_ = eig.tile([128, G], F32)
    rs_ = eig.tile([128, G], F32)
    xn2 = eig.tile([128, G], F32)
    sgn = eig.tile([128, G], F32)
    t1 = eig.tile([128, G], F32)
    beta = eig.tile([128, G], F32)
    cdot = eig.tile([128, G], F32)
    kc = eig.tile([128, G], F32)
    a_ = eig.tile([128, G, 16], F32)
    e_ = eig.tile([128, G, 16], F32)
    cc = eig.tile([128, G, 16], F32)
    V = eig.tile([128, G, 16], F32)
    W = eig.tile([128, G, 16], F32)
    WP = eig.tile([128, G, 16], F32)
    X2 = eig.tile([128, G, 16], F32)
    Pb = eig.tile([128, G, 15, 15], F32)
    Tall = eig.tile([128, G, 16], F32)

    Ev = E[:, :].rearrange("p (i g j) -> p g i j", i=16, g=G, j=16)

    # s = trace/16 ; A = C/s  (diag AP: step 17 inside each 256 block)
    diag_ap = _re(E[:, :], [[Erow, 128], [16, G], [16 * G + 1, 16]])
    nc.vector.tensor_reduce(s_[:, :], diag_ap, AX.X, ALU.add)
    nc.vector.tensor_scalar(s_[:, :], s_[:, :], 1.0 / NA, None, ALU.mult)
    nc.vector.reciprocal(rs_[:, :], s_[:, :])
    rs_b2 = rs_[:, :].unsqueeze(1).unsqueeze(3).broadcast_to([128, 16, G, 16])
    Ef = E[:, :].rearrange("p (i g j) -> p i g j", i=16, g=G, j=16)
    nc.vector.tensor_mul(Ef, Ef, rs_b2)

    # ---- Householder tridiagonalization, steps k=0..13 ----
    for k in range(NA - 2):
        L = NA - k - 1
        x_col = Ev[:, :, k + 1:, k]                       # [128, G, L]
        x0 = Ev[:, :, k + 1, k]                           # [128, G]
        ek = e_[:, :, k]                                  # [128, G] (stores -alpha)

        nc.vector.tensor_mul(X2[:, :, 0:L], x_col, x_col)
        nc.vector.tensor_reduce(xn2[:, :], X2[:, :, 0:L], AX.X, ALU.add)
        nc.scalar.sqrt(t1[:, :], xn2[:, :])               # ||x||
        nc.vector.tensor_scalar(sgn[:, :], x0, 0.0, None, ALU.is_ge)
        nc.vector.tensor_scalar(sgn[:, :], sgn[:, :], 2.0, -1.0, ALU.mult, ALU.add)
        nc.vector.tensor_mul(ek, sgn[:, :], t1[:, :])     # aneg = sign(x0)*||x||
        # denom = xn2 + x0*aneg ; beta = 1/denom (guarded)
        nc.vector.tensor_mul(t1[:, :], x0, ek)
        nc.vector.tensor_add(t1[:, :], t1[:, :], xn2[:, :])
        nc.vector.tensor_scalar_max(t1[:, :], t1[:, :], EPS_GUARD)
        nc.vector.reciprocal(beta[:, :], t1[:, :])
        # v = x + aneg*e1
        nc.vector.tensor_copy(V[:, :, 0:L], x_col)
        nc.vector.tensor_add(V[:, :, 0], V[:, :, 0], ek)
        # w = A_sub @ v
        v_bj = V[:, :, 0:L].unsqueeze(2).broadcast_to([128, G, L, L])
        nc.vector.tensor_mul(Pb[:, :, 0:L, 0:L], Ev[:, :, k + 1:, k + 1:], v_bj)
        nc.vector.tensor_reduce(W[:, :, 0:L], Pb[:, :, 0:L, 0:L], AX.X, ALU.add)
        # c = v.w ; Kc = 0.5*beta^2*c
        nc.vector.tensor_mul(X2[:, :, 0:L], V[:, :, 0:L], W[:, :, 0:L])
        nc.vector.tensor_reduce(cdot[:, :], X2[:, :, 0:L], AX.X, ALU.add)
        nc.vector.tensor_mul(kc[:, :], beta[:, :], cdot[:, :])
        nc.vector.tensor_scalar(kc[:, :], kc[:, :], 0.5, None, ALU.mult)
        nc.vector.tensor_mul(kc[:, :], kc[:, :], beta[:, :])
        # w' = beta*w - Kc*v
        beta_b = beta[:, :].unsqueeze(2).broadcast_to([128, G, L])
        kc_b = kc[:, :].unsqueeze(2).broadcast_to([128, G, L])
        nc.vector.tensor_mul(WP[:, :, 0:L], W[:, :, 0:L], beta_b)
        nc.vector.tensor_mul(X2[:, :, 0:L], V[:, :, 0:L], kc_b)
        nc.vector.tensor_sub(WP[:, :, 0:L], WP[:, :, 0:L], X2[:, :, 0:L])
        # A_sub -= v w'^T + w' v^T
        v_bi = V[:, :, 0:L].unsqueeze(3).broadcast_to([128, G, L, L])
        wp_bj = WP[:, :, 0:L].unsqueeze(2).broadcast_to([128, G, L, L])
        nc.vector.tensor_mul(Pb[:, :, 0:L, 0:L], v_bi, wp_bj)
        Asub = Ev[:, :, k + 1:, k + 1:]
        nc.vector.tensor_sub(Asub, Asub, Pb[:, :, 0:L, 0:L])
        nc.vector.tensor_sub(Asub, Asub, Pb[:, :, 0:L, 0:L].transpose([0, 1, 3, 2]))

    # gather tridiag: a = diag(A), e_[14] = A[15, 14]
    diag_ap2 = _re(E[:, :], [[Erow, 128], [16, G], [16 * G + 1, 16]])
    nc.vector.tensor_copy(a_[:, :, :], diag_ap2)
    nc.vector.tensor_copy(e_[:, :, 14], Ev[:, :, 15, 14])
    nc.vector.tensor_mul(cc[:, :, 0:15], e_[:, :, 0:15], e_[:, :, 0:15])

    # ---- Sturm grid: ok_j = [x_j < lambda_min], via sign alternation ----
    ub = t1                     # reuse [128, G]
    nc.vector.tensor_reduce(ub[:, :], a_[:, :, :], AX.X, ALU.min)
    xg = eig.tile([128, G, NX], F32)
    pp1 = eig.tile([128, G, NX], F32)
    pp2 = eig.tile([128, G, NX], F32)
    sc1 = eig.tile([128, G, NX], F32)
    ok = eig.tile([128, G, NX], F32)
    ub_b = ub[:, :].unsqueeze(2).broadcast_to([128, G, NX])
    fr_b = fr[:, :].unsqueeze(1).broadcast_to([128, G, NX])
    nc.vector.tensor_mul(xg[:, :, :], ub_b, fr_b)
    nc.vector.memset(pp2[:, :, :], 1.0)                   # p0 = 1
    a0_b = a_[:, :, 0].unsqueeze(2).broadcast_to([128, G, NX])
    nc.vector.tensor_sub(pp1[:, :, :], xg[:, :, :], a0_b)  # p1 = x - a0
    nc.vector.tensor_scalar(ok[:, :, :], pp1[:, :, :], 0.0, None, ALU.is_lt)
    for k in range(1, NA):
        ak_b = a_[:, :, k].unsqueeze(2).broadcast_to([128, G, NX])
        ck_b = cc[:, :, k - 1].unsqueeze(2).broadcast_to([128, G, NX])
        nc.vector.tensor_sub(sc1[:, :, :], xg[:, :, :], ak_b)      # t
        nc.vector.tensor_mul(sc1[:, :, :], sc1[:, :, :], pp1[:, :, :])
        nc.vector.tensor_mul(pp2[:, :, :], pp2[:, :, :], ck_b)
        nc.vector.tensor_sub(pp2[:, :, :], sc1[:, :, :], pp2[:, :, :])  # p_{k+1} into pp2
        cmp_op = ALU.is_gt if (k + 1) % 2 == 0 else ALU.is_lt
        nc.vector.tensor_scalar(sc1[:, :, :], pp2[:, :, :], 0.0, None, cmp_op)
        nc.vector.tensor_mul(ok[:, :, :], ok[:, :, :], sc1[:, :, :])
        pp1, pp2 = pp2, pp1
    xx = eig.tile([128, G], F32)
    nc.vector.tensor_reduce(xx[:, :], ok[:, :, :], AX.X, ALU.add)   # cnt
    nc.vector.tensor_mul(xx[:, :], xx[:, :], ub[:, :])
    nc.vector.tensor_scalar(xx[:, :], xx[:, :], 1.0 / (NX + 1.0), None, ALU.mult)

    # ---- Laguerre iterations on p, p', p'' ----
    S1 = eig.tile([128, 3, G], F32)
    S2 = eig.tile([128, 3, G], F32)
    R1 = eig.tile([128, 3, G], F32)
    R2 = eig.tile([128, 3, G], F32)
    msk = eig.tile([128, G], mybir.dt.uint8)
    gg = eig.tile([128, G], F32)
    hh = eig.tile([128, G], F32)
    sq = eig.tile([128, G], F32)
    d1c = eig.tile([128, G], F32)
    d2c = eig.tile([128, G], F32)
    m1 = eig.tile([128, G], F32)
    m2_ = eig.tile([128, G], F32)
    for it in range(N_LAG):
        # T_all = x - a_k for all k
        x_b = xx[:, :].unsqueeze(2).broadcast_to([128, G, 16])
        nc.vector.tensor_sub(Tall[:, :, :], x_b, a_[:, :, :])
        # init states: S1 = (p1, d1, s1) = (x-a0, 1, 0); S2 = (p0,d0,s0) = (1,0,0)
        nc.vector.tensor_copy(S1[:, 0, :], Tall[:, :, 0])
        nc.vector.memset(S1[:, 1, :], 1.0)
        nc.vector.memset(S1[:, 2, :], 0.0)
        nc.vector.memset(S2[:, 0, :], 1.0)
        nc.vector.memset(S2[:, 1:3, :], 0.0)
        cur1, cur2 = S1, S2
        for k in range(1, NA):
            t_b = Tall[:, :, k].unsqueeze(1).broadcast_to([128, 3, G])
            c_b = cc[:, :, k - 1].unsqueeze(1).broadcast_to([128, 3, G])
            nc.vector.tensor_mul(R1[:, :, :], cur1[:, :, :], t_b)
            nc.vector.tensor_mul(R2[:, :, :], cur2[:, :, :], c_b)
            nc.vector.tensor_sub(cur2[:, :, :], R1[:, :, :], R2[:, :, :])
            nc.vector.tensor_mul(R1[:, 0:2, :], cur1[:, 0:2, :], k2[:, :].rearrange("p (a g) -> p a g", a=2))
            nc.vector.tensor_add(cur2[:, 1:3, :], cur2[:, 1:3, :], R1[:, 0:2, :])
            cur1, cur2 = cur2, cur1
        # p, dp, ddp = cur1 slots; guard p away from 0
        pv = cur1[:, 0, :]
        nc.vector.tensor_scalar(m1[:, :], pv, 0.0, None, ALU.is_equal)
        nc.vector.tensor_scalar(m2_[:, :], m1[:, :], P_GUARD, None, ALU.mult)
        nc.vector.tensor_add(m1[:, :], pv, m2_[:, :])       # p_safe
        nc.vector.reciprocal(m2_[:, :], m1[:, :])           # 1/p
        nc.vector.tensor_mul(gg[:, :], cur1[:, 1, :], m2_[:, :])   # G = p'/p
        nc.vector.tensor_mul(hh[:, :], cur1[:, 2, :], m2_[:, :])   # s/p
        nc.vector.tensor_mul(m1[:, :], gg[:, :], gg[:, :])         # G^2
        nc.vector.tensor_sub(hh[:, :], m1[:, :], hh[:, :])         # H = G^2 - s/p
        # disc = (n-1)*(n*H - G^2)
        nc.vector.tensor_scalar(hh[:, :], hh[:, :], n, None, ALU.mult)
        nc.vector.tensor_sub(hh[:, :], hh[:, :], m1[:, :])
        nc.vector.tensor_scalar(hh[:, :], hh[:, :], n - 1.0, None, ALU.mult)
        nc.vector.tensor_scalar_max(hh[:, :], hh[:, :], 0.0)
        nc.scalar.sqrt(sq[:, :], hh[:, :])
        nc.vector.tensor_add(d1c[:, :], gg[:, :], sq[:, :])
        nc.vector.tensor_sub(d2c[:, :], gg[:, :], sq[:, :])
        # pick larger-magnitude denominator
        nc.scalar.activation(m1[:, :], d1c[:, :], ACTF.Abs)
        nc.scalar.activation(m2_[:, :], d2c[:, :], ACTF.Abs)
        nc.vector.tensor_tensor(msk[:, :], m1[:, :], m2_[:, :], ALU.is_ge)
        nc.vector.select(beta[:, :], msk[:, :], d1c[:, :], d2c[:, :])
        # guard |den| > 0
        nc.vector.tensor_scalar(m1[:, :], beta[:, :], 0.0, None, ALU.is_equal)
        nc.vector.tensor_scalar(m1[:, :], m1[:, :], P_GUARD, None, ALU.mult)
        nc.vector.tensor_add(beta[:, :], beta[:, :], m1[:, :])
        nc.vector.reciprocal(m2_[:, :], beta[:, :])
        nc.vector.tensor_scalar(m2_[:, :], m2_[:, :], n, None, ALU.mult)
        nc.vector.tensor_sub(xx[:, :], xx[:, :], m2_[:, :])   # x -= n/den
    # clamp to [0, inf): PSD
    nc.vector.tensor_scalar_max(xx[:, :], xx[:, :], 0.0)
    # rescale and write out
    nc.vector.tensor_mul(xx[:, :], xx[:, :], s_[:, :])
    out_view = out_ap.rearrange("(p g) -> p g", g=G)
    nc.sync.dma_start(out_view, xx[:, :])


def _build_nc(n_tiles):
    import contextlib
    from concourse import bacc
    nc = bacc.Bacc("TRN2", target_bir_lowering=False, debug=False)
    x = nc.dram_tensor("x", [n_tiles * 128, FEAT], F32, kind="ExternalInput")
    out = nc.dram_tensor("out", [n_tiles * 8], F32, kind="ExternalOutput")
    with tile.TileContext(nc) as tc:
        with contextlib.ExitStack() as ctx:
            tc._kernel_ctx = ctx
            build_core_kernel(tc, x.ap(), out.ap(), n_tiles)
    nc.compile()
    return nc


_NC_CACHE = {}


def kernel(scalar_representation, idx_m):
    """Full-input entry point: shard over 8 cores, run, gather."""
    from concourse.bass_utils import run_bass_kernel_spmd
    sr = np.ascontiguousarray(np.asarray(scalar_representation, dtype=np.float32))
    assert sr.shape == (131072, 128)
    if "nc" not in _NC_CACHE:
        _NC_CACHE["nc"] = _build_nc(N_TILES_FULL)
    nc = _NC_CACHE["nc"]
    shards = sr.reshape(N_CORES, ROWS_PER_CORE, FEAT)
    in_maps = [{"x": shards[i]} for i in range(N_CORES)]
    res = run_bass_kernel_spmd(nc, in_maps, core_ids=list(range(N_CORES)))
    out = np.concatenate([res.results[i]["out"] for i in range(N_CORES)])
    return out.astype(np.float32)


if __name__ == "__main__":
    # smoke test vs numpy on the full input
    rng = np.random.default_rng(0)
    sr = rng.standard_normal((131072, 128)).astype(np.float32)
    idx = np.repeat(np.arange(8192, dtype=np.int32), 16)
    lam = kernel(sr, idx)
    T = sr.reshape(8192, 16, 128)
    C = np.einsum('mif,mjf->mij', T, T)
    ref = np.linalg.eigvalsh(C.astype(np.float64))[:, 0]
    rel = np.abs(lam - ref) / np.abs(ref)
    print("max_rel", rel.max(), "mean_rel", rel.mean())
